# revision 1
# baseline (speedup 1.0000x reference)
"""Multi-head self-attention with LoRA projections on 8 Trainium2 NeuronCores.

Problem: nn_MultiHeadSelfAttention (B=2, L=2048, D=1024, H=16, hd=64, LoRA r=16).

Sharding: tensor-parallel on heads for QKV + attention (2 heads/core), then an
AllToAll reshards by token for the output projection (512 tokens/core), so the
final output is a clean concat (no host-side reduction).

Per-core pipeline (bf16 on the PE, fp32 accumulation in PSUM):
  1. Weight prep: W_eff = W^T + 0.5*A@B folded on-chip (W^T via PE matmul
     against identity, LoRA via K=16 matmul), cast to bf16.
     attn scale 1/sqrt(hd) folded into W_eff_q and bq.
  2. x [4096,1024] cast fp32->bf16 via SWDGE DMA (HBM->HBM), then HWDGE
     DMA-transpose into xT [8 x 128 x 4096] in SBUF.
  3. qT/kT = W_eff^T x^T in [out, token] layout (bias added during the
     PSUM->SBUF copy on ScalarE); v in [token, d] layout with a ones column
     appended (softmax row-sums); bv deferred past softmax (rows of softmax
     sum to 1, so P@(v + 1 bv^T) = P@v + bv).
  4. Per (batch, head): S^T tiles [m=128, l=512] in PSUM; attn bias is
     cast-DMA'd to bf16 natural [l, m] and injected into PSUM by PE matmuls
     against identity (= transpose), scores accumulate on top; exp on ScalarE
     -> E^T bf16; AV matmul with lhsT=[v|1] accumulates O'^T [65, l] where
     row 64 is the softmax denominator; finalize: transpose O'^T tiles,
     normalize (DVE reciprocal + per-partition scale), transpose back to
     OT [head_dim, token] adding bv.
  5. AllToAll (one per batch) reshards OT from head-split to token-split.
  6. Output projection y = OT_full^T @ Wo_eff + bo for this core's 512 tokens.

Host side only shards/gathers: slices weights/bias per core, concatenates the
per-core [512, 1024] outputs.
"""

import numpy as np

B = 2
L = 2048
D = 1024
H = 16
HD = 64
R = 16
SCALING = 0.5  # LoRA alpha/r
SCALE = HD ** (-0.5)  # attention scale, folded into Wq_eff/bq

N_CORES = 8
HPC = H // N_CORES  # heads per core = 2
OPC = HPC * HD  # out-dims per core for q/k/v = 128
TOK = B * L  # 4096
TPC = TOK // N_CORES  # tokens per core after AllToAll = 512
TPB = L // N_CORES  # tokens per (core, batch) = 256

_CACHE = {}


def _build_kernel(no_collective=False, num_devices=N_CORES, repeat=1):
    import concourse.tile as tile
    import concourse.mybir as mybir
    from concourse import bacc
    from concourse.masks import make_identity
    from contextlib import ExitStack

    f32 = mybir.dt.float32
    bf16 = mybir.dt.bfloat16
    AF = mybir.ActivationFunctionType

    nc = bacc.Bacc("TRN2", target_bir_lowering=False, debug=False,
                   enable_asserts=False, num_devices=num_devices)

    # ---- per-core external inputs ----
    x_ap = nc.dram_tensor("x", [TOK, D], f32, kind="ExternalInput").ap()
    bias_ap = nc.dram_tensor("bias", [HPC, L, L], f32, kind="ExternalInput").ap()
    w_aps, b_aps, a_aps, lb_aps = {}, {}, {}, {}
    for p in "qkv":
        w_aps[p] = nc.dram_tensor(f"W{p}", [OPC, D], f32, kind="ExternalInput").ap()
        b_aps[p] = nc.dram_tensor(f"b{p}", [OPC, 1], f32, kind="ExternalInput").ap()
        a_aps[p] = nc.dram_tensor(f"A{p}", [D, R], f32, kind="ExternalInput").ap()
        lb_aps[p] = nc.dram_tensor(f"B{p}", [R, OPC], f32, kind="ExternalInput").ap()
    wo_ap = nc.dram_tensor("Wo", [D, D], f32, kind="ExternalInput").ap()
    bo_ap = nc.dram_tensor("bo", [1, D], f32, kind="ExternalInput").ap()
    ao_ap = nc.dram_tensor("Ao", [D, R], f32, kind="ExternalInput").ap()
    lbo_ap = nc.dram_tensor("Bo", [R, D], f32, kind="ExternalInput").ap()

    y_ap = nc.dram_tensor("y", [TPC, D], f32, kind="ExternalOutput").ap()

    KT = D // 128  # 8 contraction k-tiles for the projections
    LT = L // 128  # 16 l-tiles per batch
    MT = L // 128  # 16 m-tiles per batch
    LCH = 1024  # l-chunk (2 PSUM banks; exp runs as one wide ACT op)
    NLC = L // LCH  # 2 l-chunks per batch
    MC = 512  # m-chunk for staged bias
    NMC = L // MC  # 4 m-chunks

    with tile.TileContext(nc) as tc, ExitStack() as top:
        const_pool = top.enter_context(tc.tile_pool(name="const", bufs=1))
        ident = const_pool.tile([128, 128], bf16)
        make_identity(nc, ident[:])
        identf = const_pool.tile([128, 128], f32)
        make_identity(nc, identf[:])
        ones_row = const_pool.tile([1, 128], bf16)
        nc.gpsimd.memset(ones_row[:], 1.0)

        dram = top.enter_context(tc.tile_pool(name="dram", bufs=1, space="DRAM"))

        for rep in range(repeat):
          with ExitStack() as rctx:
            # ================= weight prep =================
            weff_pool = rctx.enter_context(tc.tile_pool(name="weff", bufs=1))
            weff = {p: weff_pool.tile([128, D], bf16, name=f"weff_{p}") for p in "qkv"}
            woeff = weff_pool.tile([128, KT, D], bf16)  # [hd in tile, ktile, out]
            bias_q = const_pool.tile([128, 1], f32)
            bias_k = const_pool.tile([128, 1], f32)
            bv_cat = const_pool.tile([128, 1], f32)
            bo_row = const_pool.tile([1, D], bf16)

            early_bias = rctx.enter_context(tc.tile_pool(name="early_bias", bufs=4))
            qkv_pool = rctx.enter_context(tc.tile_pool(name="qkv", bufs=1))
            qT = qkv_pool.tile([128, TOK], bf16)
            kT = qkv_pool.tile([128, TOK], bf16)
            vsb = qkv_pool.tile([128, TOK // 128, 130], bf16)

            with ExitStack() as xctx:
                xp = xctx.enter_context(tc.tile_pool(name="xT", bufs=1))
                xT = xp.tile([128, KT, TOK], bf16)
                with ExitStack() as wctx:
                    wsm = wctx.enter_context(tc.tile_pool(name="wsm", bufs=4))
                    won_pool = wctx.enter_context(tc.tile_pool(name="won", bufs=1))
                    wps = wctx.enter_context(
                        tc.tile_pool(name="wps", bufs=1, space="PSUM"))

                    # ---- all weight DMAs up front (SWDGE queue fills early) ----
                    wnat, anat, lb = {}, {}, {}
                    for p in "qkv":
                        wnat[p] = wsm.tile([128, D], bf16, tag="wnat", name=f"wn{p}")
                        nc.gpsimd.dma_start(wnat[p][:], w_aps[p][:, :])
                        anat[p] = wsm.tile([128, KT, R], bf16, tag="anat",
                                           name=f"an{p}")
                        nc.gpsimd.dma_start(
                            anat[p][:],
                            a_aps[p].rearrange("(kt p2) r -> p2 kt r", p2=128))
                        lb[p] = wsm.tile([R, 128], bf16, tag="lb", name=f"lb{p}")
                        nc.gpsimd.dma_start(lb[p][:], lb_aps[p][:, :])
                    anat["o"] = wsm.tile([128, KT, R], bf16, tag="anat", name="ano")
                    nc.gpsimd.dma_start(
                        anat["o"][:], ao_ap.rearrange("(kt p2) r -> p2 kt r", p2=128))
                    lbo = wsm.tile([R, D], bf16, tag="lbo")
                    nc.gpsimd.dma_start(lbo[:], lbo_ap[:, :])
                    wonat = won_pool.tile([128, KT, D], bf16)
                    nc.gpsimd.dma_start(
                        wonat[:], wo_ap.rearrange("(oj p2) d -> p2 oj d", p2=128))
                    nc.gpsimd.dma_start(bo_row[:], bo_ap[:, :])  # cast to bf16
                    bq_raw = wsm.tile([128, 1], f32, tag="braw")
                    nc.sync.dma_start(bq_raw[:], b_aps["q"][:, :])
                    nc.sync.dma_start(bias_k[:], b_aps["k"][:, :])
                    nc.sync.dma_start(bv_cat[:], b_aps["v"][:, :])
                    nc.gpsimd.memset(vsb[:, :, 64:65], 1.0)
                    nc.gpsimd.memset(vsb[:, :, 129:130], 1.0)
                    nc.gpsimd.memset(ones_row[:], 1.0)
                    # prefetch h0's first bias chunks during the x/qkv phase
                    MH = 256
                    early_bstage = []
                    for mh in range(4):
                        bt = early_bias.tile([128, LT, MH], bf16, tag="eb",
                                             name=f"eb{mh}")
                        nc.gpsimd.dma_start(
                            bt[:],
                            bias_ap[0].rearrange("(lt p) m -> p lt m", p=128)[
                                :, :, mh * MH:(mh + 1) * MH])
                        early_bstage.append(bt)

                    # ---- weight-prep compute pieces (interleaved into x loop) ----
                    ats = {}

                    def piece_at(p):
                        def run():
                            pat = wps.tile([R, D], bf16, tag="wps", name=f"pat{p}")
                            for kt2 in range(KT):
                                nc.tensor.matmul(pat[:, kt2 * 128:(kt2 + 1) * 128],
                                                 anat[p][:, kt2, :], ident[:],
                                                 is_transpose=True)
                            ats[p] = wsm.tile([R, D], bf16, tag="ats", name=f"at{p}")
                            nc.vector.tensor_scalar_mul(ats[p][:], pat[:], SCALING)
                        return run

                    def piece_proj(p):
                        def run():
                            ps = wps.tile([128, D], f32, tag="wps", name=f"wq{p}")
                            for ki in range(KT):
                                ksl = slice(ki * 128, (ki + 1) * 128)
                                nc.tensor.matmul(ps[:, ksl], wnat[p][:, ksl],
                                                 ident[:], start=True, stop=False,
                                                 skip_group_check=True)
                                nc.tensor.matmul(ps[:, ksl], ats[p][:, ksl],
                                                 lb[p][:], start=False, stop=True,
                                                 skip_group_check=True)
                            sc = SCALE if p == "q" else 1.0
                            nc.vector.tensor_scalar_mul(weff[p][:], ps[:], sc)
                        return run

                    def piece_wo(ki):
                        def run():
                            ksl = slice(ki * 128, (ki + 1) * 128)
                            ps = wps.tile([128, D], f32, tag="wps", name=f"wo{ki}")
                            for oj in range(KT):
                                osl2 = slice(oj * 128, (oj + 1) * 128)
                                nc.tensor.matmul(ps[:, osl2], wonat[:, oj, ksl],
                                                 ident[:], start=True, stop=False,
                                                 skip_group_check=True)
                                nc.tensor.matmul(ps[:, osl2], ats["o"][:, ksl],
                                                 lbo[:, osl2], start=False,
                                                 stop=True, skip_group_check=True)
                            nc.vector.tensor_copy(woeff[:, ki, :], ps[:])
                        return run

                    def piece_bias():
                        def run():
                            nc.vector.tensor_scalar_mul(bias_q[:], bq_raw[:], SCALE)
                        return run

                    # ---- interleaved x-transpose / weight-prep / q,k,v projections ----
                    pp = wctx.enter_context(tc.tile_pool(name="proj_ps", bufs=3,
                                                         space="PSUM"))

                    def qk_chunk(lc):
                        tsl = slice(lc * 512, (lc + 1) * 512)
                        for dst, p, bias_t in ((qT, "q", bias_q), (kT, "k", bias_k)):
                            ps = pp.tile([128, 512], f32, tag="proj",
                                         name=f"pj{p}{lc}")
                            for ki in range(KT):
                                ksl = slice(ki * 128, (ki + 1) * 128)
                                nc.tensor.matmul(ps[:], weff[p][:, ksl],
                                                 xT[:, ki, tsl],
                                                 start=(ki == 0), stop=(ki == KT - 1))
                            nc.vector.tensor_scalar_add(dst[:, tsl], ps[:], bias_t[:])

                    def v_tile(tt):
                        tsl = slice(tt * 128, (tt + 1) * 128)
                        ps = pp.tile([128, 128], f32, tag="proj", name=f"pv{tt}")
                        for ki in range(KT):
                            ksl = slice(ki * 128, (ki + 1) * 128)
                            nc.tensor.matmul(ps[:], xT[:, ki, tsl], weff["v"][:, ksl],
                                             start=(ki == 0), stop=(ki == KT - 1))
                        nc.vector.tensor_copy(vsb[:, tt, 0:64], ps[:, 0:64])
                        nc.vector.tensor_copy(vsb[:, tt, 65:129], ps[:, 64:128])

                    with ExitStack() as xtctx:
                        xload = xtctx.enter_context(
                            tc.tile_pool(name="xload", bufs=3))
                        xps = xtctx.enter_context(
                            tc.tile_pool(name="xps", bufs=2, space="PSUM"))
                        XCH = 2  # token-tiles per cast-DMA chunk
                        NTT = TOK // 128

                        # pieces needed early (weff q/k/v); o/wo/bias deferred
                        early_pieces = {4: piece_at("q"), 5: piece_proj("q"),
                                        6: piece_at("k"), 7: piece_proj("k"),
                                        8: piece_at("v"), 9: piece_proj("v"),
                                        10: piece_bias()}
                        late_pieces = ([piece_at("o")]
                                       + [piece_wo(ki) for ki in range(KT)])
                        qk_pending = list(range(TOK // 512))  # 8 chunks
                        v_pending = list(range(NTT))

                        def fill(tt):
                            # one projection per x-tile slot, once inputs exist
                            if tt < 11:
                                return
                            if qk_pending and qk_pending[0] * 4 + 3 <= tt:
                                qk_chunk(qk_pending.pop(0))
                            elif v_pending and v_pending[0] <= tt:
                                v_tile(v_pending.pop(0))

                        for tt in range(NTT):
                            tc_, to = divmod(tt, XCH)
                            if to == 0:
                                xn = xload.tile([128, XCH, D], bf16, tag="xn")
                                nc.gpsimd.dma_start(
                                    xn[:],
                                    x_ap[tc_ * XCH * 128:(tc_ + 1) * XCH * 128, :]
                                    .rearrange("(c p2) d -> p2 c d", p2=128))
                            xq = xps.tile([128, D], bf16, tag="xq")
                            for ki in range(KT):
                                ksl = slice(ki * 128, (ki + 1) * 128)
                                nc.tensor.matmul(xq[:, ksl], xn[:, to, ksl], ident[:],
                                                 is_transpose=True)
                            nc.vector.tensor_copy(
                                xT[:, :, tt * 128:(tt + 1) * 128],
                                xq[:].rearrange("p (ki t) -> p ki t", ki=KT))
                            if tt in early_pieces:
                                early_pieces[tt]()
                            else:
                                fill(tt)
                        # drain remaining projections, interleaving the deferred
                        # o-projection weight prep
                        while qk_pending or v_pending or late_pieces:
                            if qk_pending:
                                qk_chunk(qk_pending.pop(0))
                            elif late_pieces:
                                late_pieces.pop(0)()
                            for _ in range(2):
                                if v_pending:
                                    v_tile(v_pending.pop(0))

            # ================= attention =================
            ot_pool = rctx.enter_context(tc.tile_pool(name="ot", bufs=2))
            a2a_pool = rctx.enter_context(tc.tile_pool(name="a2a", bufs=2, space="DRAM"))
            y_pool = rctx.enter_context(tc.tile_pool(name="ysb", bufs=2))
            otf_pool = rctx.enter_context(tc.tile_pool(name="otf", bufs=2))

            with ExitStack() as actx:
                bias_pool = actx.enter_context(tc.tile_pool(name="bias_nat", bufs=8))
                psA = actx.enter_context(tc.tile_pool(name="psA", bufs=3, space="PSUM"))
                psB = actx.enter_context(tc.tile_pool(name="psB", bufs=1, space="PSUM"))
                e_pool = actx.enter_context(tc.tile_pool(name="e", bufs=3))
                fin_pool = actx.enter_context(tc.tile_pool(name="fin", bufs=2))
                ocat_pool = actx.enter_context(tc.tile_pool(name="ocat", bufs=2))

                ocats = [ocat_pool.tile([128, LT, 128], bf16, tag="ocat",
                                        name=f"ocat{bx}") for bx in range(B)]

                def attention(h, lc, b, bstage):
                    # one l-chunk of one batch: S^T/exp/AV over all m, then
                    # normalize into ocat rows lc*8..lc*8+7
                    hsl = slice(h * 64, (h + 1) * 64)
                    lof = b * L + lc * LCH
                    qTh = qT[hsl, lof:lof + LCH]
                    kTh = kT[hsl, b * L:(b + 1) * L]
                    po = psB.tile([65, LCH], f32, tag="po", name=f"po{h}{b}{lc}")
                    for mi in range(MT):
                        mh, mo = divmod(mi * 128, MH)
                        ps = psA.tile([128, LCH], f32, tag="ps")
                        for half in range(2):
                            hof = half * 512
                            nc.tensor.matmul(
                                ps[:, hof:hof + 512],
                                kTh[:, mi * 128:(mi + 1) * 128],
                                qTh[:, hof:hof + 512],
                                start=True, stop=False, skip_group_check=True)
                            for j in range(4):
                                lt = lc * (LCH // 128) + half * 4 + j
                                nc.tensor.matmul(
                                    ps[:, hof + j * 128:hof + (j + 1) * 128],
                                    bstage[mh][:, lt, mo:mo + 128],
                                    ident[:], start=False,
                                    stop=(j == 3), skip_group_check=True)
                        e = e_pool.tile([128, LCH], bf16, tag="e")
                        nc.scalar.activation(e[:], ps[:], AF.Exp)
                        for half in range(2):
                            nc.tensor.matmul(
                                po[:, half * 512:(half + 1) * 512],
                                vsb[:, b * MT + mi, h * 65:h * 65 + 65],
                                e[:, half * 512:(half + 1) * 512],
                                start=(mi == 0), stop=(mi == MT - 1),
                                skip_group_check=True)
                    # copy out of PSUM quickly (frees po), then normalize
                    # (split across DVE+ACT so the single po bank frees sooner)
                    stage = fin_pool.tile([65, LCH], f32, tag="st")
                    nc.vector.tensor_copy(stage[:, 0:512], po[:, 0:512])
                    nc.scalar.copy(stage[:, 512:1024], po[:, 512:1024])
                    for j in range(LCH // 128):
                        lt = lc * (LCH // 128) + j
                        pf = psA.tile([128, 65], f32, tag="ps", name=f"pf{h}{b}{lt}")
                        nc.tensor.matmul(pf[:], stage[:, j * 128:(j + 1) * 128],
                                         identf[0:65, 0:65], is_transpose=True)
                        rec = fin_pool.tile([128, 1], f32, tag="rec")
                        nc.vector.reciprocal(rec[:], pf[:, 64:65])
                        nc.vector.tensor_scalar_mul(
                            ocats[b][:, lt, hsl], pf[:, 0:64], rec[:])

                otfs = {}
                a2a_in = a2a_pool.tile([N_CORES, 128, B, TPB], bf16, tag="ain")
                a2a_out = a2a_pool.tile([N_CORES, 128, B, TPB], bf16, tag="aout")

                def finish_comm(b):
                    # transpose ocat -> OT [hd 128, l] and add bv; stage into
                    # this batch's half of the (single, merged) AllToAll buffer
                    ot = ot_pool.tile([128, L], bf16, tag="ot")
                    for lt in range(LT):
                        pt = psA.tile([128, 128], bf16, tag="ps", name=f"pt{b}{lt}")
                        nc.tensor.matmul(pt[:], ocats[b][:, lt, :], ident[:],
                                         is_transpose=True)
                        nc.vector.tensor_scalar_add(
                            ot[:, lt * 128:(lt + 1) * 128], pt[:], bv_cat[:])
                    nc.sync.dma_start(
                        a2a_in[:, :, b, :].rearrange("j p t -> p j t"), ot[:])

                def finish_a2a():
                    # single AllToAll: head-split -> token-split, both batches
                    if no_collective:
                        nc.sync.dma_start(a2a_out[:], a2a_in[:])
                    else:
                        nc.gpsimd.collective_compute(
                            "AllToAll", mybir.AluOpType.bypass,
                            replica_groups=[list(range(N_CORES))],
                            ins=[a2a_in.opt()], outs=[a2a_out.opt()])
                    otf = otf_pool.tile([128, N_CORES, B, TPB], bf16, tag="otf")
                    nc.sync.dma_start(otf[:],
                                      a2a_out[:].rearrange("j p b t -> p j b t"))
                    otfs[0] = otf

                def finish_oproj(b):
                    otf = otfs[0]
                    # ---- output projection for this batch's 256 tokens ----
                    for tt in range(TPB // 128):
                        tsl = slice(tt * 128, (tt + 1) * 128)
                        for nch in range(2):
                            nsl = slice(nch * 512, (nch + 1) * 512)
                            ps = psA.tile([128, 512], f32, tag="ps",
                                          name=f"psy{b}{tt}{nch}")
                            for ki in range(KT):
                                nc.tensor.matmul(
                                    ps[:], otf[:, ki, b, tsl], woeff[:, ki, nsl],
                                    start=(ki == 0), stop=False, skip_group_check=True)
                            nc.tensor.matmul(ps[:], ones_row[:], bo_row[:, nsl],
                                             start=False, stop=True,
                                             skip_group_check=True)
                            ysb = y_pool.tile([128, 512], f32, tag="y")
                            nc.vector.tensor_copy(ysb[:], ps[:])
                            nc.sync.dma_start(
                                y_ap[b * TPB + tt * 128: b * TPB + (tt + 1) * 128, nsl],
                                ysb[:])

                for h in range(HPC):
                    bstage = list(early_bstage) if h == 0 else []
                    for mh in range(len(bstage), L // MH):
                        bt = bias_pool.tile([128, LT, MH], bf16, tag="bn",
                                            name=f"bn{h}{mh}")
                        nc.gpsimd.dma_start(
                            bt[:],
                            bias_ap[h].rearrange("(lt p) m -> p lt m", p=128)[
                                :, :, mh * MH:(mh + 1) * MH],
                        )
                        bstage.append(bt)
                    last = (h == HPC - 1)
                    for lc in range(NLC):
                        for b in range(B):
                            if last and lc == NLC - 1 and b == 1:
                                # batch 0 fully done: stage its A2A half early
                                finish_comm(0)
                            attention(h, lc, b, bstage)
                finish_comm(1)
                finish_a2a()
                finish_oproj(0)
                finish_oproj(1)

    nc.compile()
    return nc


def _shard_inputs(inputs):
    x = np.ascontiguousarray(inputs["x"].reshape(TOK, D))
    attn_bias = inputs["attn_bias"]
    in_maps = []
    for c in range(N_CORES):
        hsl = slice(c * HPC, (c + 1) * HPC)
        osl = slice(c * OPC, (c + 1) * OPC)
        m = {
            "x": x,
            "bias": attn_bias[0, hsl],
            "Wo": inputs["Wo"],
            "bo": inputs["bo"][None, :],
            "Ao": inputs["Ao"],
            "Bo": inputs["Bo"],
        }
        for p in "qkv":
            m[f"W{p}"] = inputs[f"W{p}"][osl]
            m[f"b{p}"] = inputs[f"b{p}"][osl][:, None]
            m[f"A{p}"] = inputs[f"A{p}"]
            m[f"B{p}"] = inputs[f"B{p}"][:, osl]
        in_maps.append(m)
    return in_maps


def _gather_outputs(results):
    y = np.empty((B, L, D), np.float32)
    for c in range(N_CORES):
        yc = results[c]["y"]
        for b in range(B):
            y[b, c * TPB:(c + 1) * TPB] = yc[b * TPB:(b + 1) * TPB]
    return y


def get_nc(**kw):
    key = ("nc", tuple(sorted(kw.items())))
    if key not in _CACHE:
        _CACHE[key] = _build_kernel(**kw)
    return _CACHE[key]


def _get_runner():
    """Cached jitted SPMD executable (avoids re-tracing on repeated calls)."""
    if "runner" in _CACHE:
        return _CACHE["runner"]
    import jax
    from jax.sharding import Mesh, PartitionSpec
    from jax.experimental.shard_map import shard_map
    import concourse.mybir as mybir
    from concourse.bass2jax import (_bass_exec_p, install_neuronx_cc_hook,
                                    partition_id_tensor)

    nc = get_nc()
    install_neuronx_cc_hook()
    partition_name = nc.partition_id_tensor.name if nc.partition_id_tensor else None
    in_names, out_names, out_avals, zero_outs = [], [], [], []
    for alloc in nc.m.functions[0].allocations:
        if not isinstance(alloc, mybir.MemoryLocationSet):
            continue
        name = alloc.memorylocations[0].name
        if alloc.kind == "ExternalInput":
            if name != partition_name:
                in_names.append(name)
        elif alloc.kind == "ExternalOutput":
            shape = tuple(alloc.tensor_shape)
            dtype = mybir.dt.np(alloc.dtype)
            out_names.append(name)
            out_avals.append(jax.core.ShapedArray(shape, dtype))
            zero_outs.append(np.zeros(shape, dtype))
    n_params = len(in_names)
    n_outs = len(out_avals)
    all_in_names = list(in_names) + list(out_names)
    if partition_name is not None:
        all_in_names.append(partition_name)

    def _body(*args):
        operands = list(args)
        if partition_name is not None:
            operands.append(partition_id_tensor())
        outs = _bass_exec_p.bind(
            *operands,
            out_avals=tuple(out_avals),
            in_names=tuple(all_in_names),
            out_names=tuple(out_names),
            lowering_input_output_aliases=(),
            sim_require_finite=True,
            sim_require_nnan=True,
            nc=nc,
        )
        return tuple(outs)

    devices = jax.devices()[:N_CORES]
    mesh = Mesh(np.asarray(devices), ("core",))
    in_specs = (PartitionSpec("core"),) * (n_params + n_outs)
    out_specs = (PartitionSpec("core"),) * n_outs
    fn = jax.jit(shard_map(_body, mesh=mesh, in_specs=in_specs,
                           out_specs=out_specs, check_rep=False),
                 keep_unused=True)
    _CACHE["runner"] = (fn, in_names, out_names, zero_outs)
    return _CACHE["runner"]


def run_on_device(in_maps):
    import jax
    fn, in_names, out_names, zero_outs = _get_runner()
    concat_in = [np.concatenate([np.asarray(in_maps[c][nm])
                                 for c in range(N_CORES)], axis=0)
                 for nm in in_names]
    concat_zeros = [np.zeros((N_CORES * z.shape[0], *z.shape[1:]), z.dtype)
                    for z in zero_outs]
    out = fn(*concat_in, *concat_zeros)
    jax.block_until_ready(out)
    results = []
    for c in range(N_CORES):
        d = {}
        for i, nm in enumerate(out_names):
            arr = np.asarray(out[i])
            per = arr.shape[0] // N_CORES
            d[nm] = arr[c * per:(c + 1) * per]
        results.append(d)
    return results


def kernel(**inputs) -> np.ndarray:
    in_maps = _shard_inputs(inputs)
    results = run_on_device(in_maps)
    return _gather_outputs(results)



# revision 39
# speedup vs baseline: 15808.9205x; 15808.9205x over previous
"""Multi-head self-attention with LoRA projections on 8 Trainium2 NeuronCores.

Problem: nn_MultiHeadSelfAttention (B=2, L=2048, D=1024, H=16, hd=64, LoRA r=16).

Sharding (communication-free): core c owns batch b = c//4 and query rows
qo = (c%4)*512 of that batch. Each core computes K/V projections for its
whole batch (replicated across the 4 cores sharing the batch), Q for its
512 rows, attention for all 16 heads over those rows, and the output
projection for those rows. The final output is a clean concat across cores —
no collectives and no host-side reduction.

Host-side prep is layout/dtype marshaling only: inputs are cast to bf16
(numerically identical to the on-device cast-DMA the previous version used),
W is shipped pre-transposed as [in, out], and attn_bias is shipped
pre-transposed per head as [key m, query l] so it can be injected into the
S^T PSUM accumulation by a single stationary-identity matmul per tile.
LoRA folding (W_eff = W^T + 0.5*A@B) happens on device.

Per-core pipeline (bf16 on the PE, fp32 accumulation in PSUM):
  1. W_eff[p] [128, 8, 1024] (in-chunk part, ki, out) loaded by DMA; LoRA
     term 0.5*A@B added via K=16 matmuls into PSUM + DVE add.
  2. x tiles PE-transposed into xT chunks [128, 8, 1024] (2 chunks of batch
     tokens for K/V, then this core's 512 query tokens reusing chunk 0's
     buffer).
  3. kT [128, 8, 2048] / qT [128, 8, 512] in [out, token] layout (+bias;
     attn scale folded into the Q PSUM->SBUF copy); v natural [token, d]
     in vsb with a ones column per head (softmax row sums); bv deferred
     past softmax (softmax rows sum to 1).
  4. Per head: S^T tiles [m=128, l=512] in PSUM (K=64 matmul), pre-transposed
     bias injected by matmul(ident, biasT_tile); exp on ScalarE -> E^T bf16;
     AV with lhsT=[v|1] accumulates O'^T [65, 512] (row 64 = denominator);
     normalize via [65]->[128] PE transpose + DVE reciprocal into ocat.
  5. OT [128, 8, 512] (d, token) via PE transposes (+bv); output projection
     y = OT^T @ Wo_eff + bo for this core's 512 rows.
"""

import numpy as np

B = 2
L = 2048
D = 1024
H = 16
HD = 64
R = 16
SCALING = 0.5  # LoRA alpha/r
SCALE = HD ** (-0.5)  # attention scale, applied in the Q PSUM->SBUF copy

N_CORES = 8
TQ = 512  # query rows per core
LT = TQ // 128  # 4 query tiles
MT = L // 128  # 16 key tiles
KT = D // 128  # 8 contraction chunks
NTT = L // 128  # 16 batch-token tiles
QTT = TQ // 128  # 4 query-token tiles

_CACHE = {}


def _build_kernel(num_devices=N_CORES, repeat=1):
    import concourse.tile as tile
    import concourse.mybir as mybir
    from concourse import bacc
    from concourse.masks import make_identity
    from contextlib import ExitStack

    f32 = mybir.dt.float32
    bf16 = mybir.dt.bfloat16
    AF = mybir.ActivationFunctionType
    ALU = mybir.AluOpType

    nc = bacc.Bacc("TRN2", target_bir_lowering=False, debug=False,
                   enable_asserts=False, num_devices=num_devices)

    # ---- per-core external inputs (bf16 unless noted) ----
    xbT_ap = nc.dram_tensor("xbT", [D, L], bf16, kind="ExternalInput").ap()
    xqT_ap = nc.dram_tensor("xqT", [D, TQ], bf16, kind="ExternalInput").ap()
    biasT_ap = nc.dram_tensor("biasT", [H, L, TQ], bf16, kind="ExternalInput").ap()
    wt_aps = {}
    for p in "qkvo":
        wt_aps[p] = nc.dram_tensor(f"WT{p}", [D, D], bf16, kind="ExternalInput").ap()
    # LoRA factors packed into 32 partition rows (base partition 0) with the
    # projection selected by free-dim blocks + zero padding:
    #   ATs[0:16, 0] = Aq^T, ATs[16:32, 0] = Ak^T,
    #   ATs[0:16, 1] = Av^T, ATs[16:32, 1] = Ao^T;
    #   Bs[:, i] = B of projection i ("qkvo") in the row range matching its
    #   A block, zeros elsewhere — so a K=32 matmul picks out one projection.
    ats_ap = nc.dram_tensor("ATs", [32, 2, D], bf16, kind="ExternalInput").ap()
    bs_ap = nc.dram_tensor("Bs", [32, 4, D], bf16, kind="ExternalInput").ap()
    bq_ap = nc.dram_tensor("bq", [D, 1], f32, kind="ExternalInput").ap()
    bk_ap = nc.dram_tensor("bk", [D, 1], f32, kind="ExternalInput").ap()
    bv_ap = nc.dram_tensor("bv", [D, 1], f32, kind="ExternalInput").ap()
    bo_ap = nc.dram_tensor("bo", [1, D], f32, kind="ExternalInput").ap()

    y_ap = nc.dram_tensor("y", [TQ, D], f32, kind="ExternalOutput").ap()

    with tile.TileContext(nc) as tc, ExitStack() as top:
        const_pool = top.enter_context(tc.tile_pool(name="const", bufs=1))
        ident = const_pool.tile([128, 128], bf16)
        make_identity(nc, ident[:])
        identf = const_pool.tile([128, 128], f32)
        make_identity(nc, identf[:])
        ones_row = const_pool.tile([1, 128], bf16)
        nc.gpsimd.memset(ones_row[:], 1.0)

        for rep in range(repeat):
          with ExitStack() as rctx:
            cpool = rctx.enter_context(tc.tile_pool(name="cvec", bufs=1))
            bq_sb = cpool.tile([128, KT], f32)
            bk_sb = cpool.tile([128, KT], f32)
            bv_sb = cpool.tile([128, KT], f32)
            bo_row = cpool.tile([1, D], bf16)
            nc.scalar.dma_start(bq_sb[:], bq_ap.rearrange("(oj p) one -> p (oj one)", p=128))
            nc.scalar.dma_start(bk_sb[:], bk_ap.rearrange("(oj p) one -> p (oj one)", p=128))
            nc.scalar.dma_start(bv_sb[:], bv_ap.rearrange("(oj p) one -> p (oj one)", p=128))

            # ---- persistent tiles for this rep ----
            kqv_pool = rctx.enter_context(tc.tile_pool(name="kqv", bufs=1))
            kT = kqv_pool.tile([128, KT, L], bf16)
            qT = kqv_pool.tile([128, KT, TQ], bf16)
            vsb = kqv_pool.tile([128, NTT, H * 65], bf16)
            woeff_pool = rctx.enter_context(tc.tile_pool(name="woeff", bufs=1))
            woeff = woeff_pool.tile([128, KT, D], bf16)
            for h in range(H):
                nc.vector.memset(vsb[:, :, h * 65 + 64:h * 65 + 65], 1.0)

            # attn bias staged per (head, half): [128, 8 m-tiles, 512]
            bias_pool = rctx.enter_context(tc.tile_pool(name="biasT", bufs=3))
            bstages = {}

            def bias_dma(h, half):
                bt = bias_pool.tile([128, MT // 2, TQ], bf16, tag="bT",
                                    name=f"bT{h}_{half}")
                nc.sync.dma_start(
                    bt[:],
                    biasT_ap[h][half * 1024:(half + 1) * 1024, :].rearrange(
                        "(mt p) l -> p mt l", p=128))
                bstages[(h, half)] = bt

            with ExitStack() as wctx:
                lora_sm = wctx.enter_context(tc.tile_pool(name="lora_sm", bufs=1))
                wps = wctx.enter_context(
                    tc.tile_pool(name="wps", bufs=2, space="PSUM"))
                weff_pool = wctx.enter_context(tc.tile_pool(name="weff", bufs=2))

                ats_raw = lora_sm.tile([32, 2, D], bf16)
                bs = lora_sm.tile([32, 4, D], bf16)
                ats = lora_sm.tile([32, 2, D], bf16)
                nc.sync.dma_start(ats_raw[:], ats_ap[:, :, :])
                nc.sync.dma_start(bs[:], bs_ap[:, :, :])
                nc.vector.tensor_scalar_mul(ats[:], ats_raw[:], SCALING)
                PI = {"q": 0, "k": 1, "v": 2, "o": 3}

                weff = {}
                weff["o"] = woeff

                def weff_dma(p, sliced=False):
                    if p != "o":
                        weff[p] = weff_pool.tile([128, KT, D], bf16, tag="weff",
                                                 name=f"weff_{p}")
                    src = wt_aps[p].rearrange("(ki p2) o -> p2 ki o", p2=128)
                    if sliced:
                        # per-ki slices so the LoRA adds pipeline behind the DMA
                        for ki in range(KT):
                            nc.sync.dma_start(
                                weff[p][:, ki:ki + 1, :], src[:, ki:ki + 1, :])
                    else:
                        nc.sync.dma_start(weff[p][:], src)

                def piece_lora(p, ki):
                    # weff[p][:, ki, :] += SCALING * A[ki-chunk, :] @ B
                    # (K=32 matmul; zero rows in the Bs block mask the paired
                    # projection sharing the lhsT rows)
                    pi = PI[p]
                    ksl = slice(ki * 128, (ki + 1) * 128)
                    ps = wps.tile([128, D], f32, tag="wps", name=f"lo{p}{ki}")
                    for half in range(2):
                        osl = slice(half * 512, (half + 1) * 512)
                        nc.tensor.matmul(ps[:, osl], ats[:, pi // 2, ksl],
                                         bs[:, pi, osl],
                                         start=True, stop=True,
                                         skip_group_check=True)
                    nc.vector.tensor_tensor(weff[p][:, ki, :], weff[p][:, ki, :],
                                            ps[:], ALU.add)

                # ---- x chunks DMA'd directly (host ships x pre-transposed) --
                xt_pool = wctx.enter_context(tc.tile_pool(name="xT", bufs=2))
                xchunks = {}

                def x_dma(cidx, half):
                    # chunk 0: batch tokens 0-1023, chunk 1: 1024-2047,
                    # chunk 2: the core's 512 query tokens (reuses buffer 0)
                    if cidx == 2:
                        xchunks[2] = xt_pool.tile([128, KT, TQ], bf16,
                                                  tag="xc", name="xc2")
                        nc.sync.dma_start(
                            xchunks[2][:],
                            xqT_ap.rearrange("(ki p) t -> p ki t", p=128))
                        return
                    if half == 0:
                        xchunks[cidx] = xt_pool.tile([128, KT, 1024], bf16,
                                                     tag="xc", name=f"xc{cidx}")
                    tof = cidx * 1024 + half * 512
                    nc.sync.dma_start(
                        xchunks[cidx][:, :, half * 512:(half + 1) * 512],
                        xbT_ap[:, tof:tof + 512].rearrange(
                            "(ki p) t -> p ki t", p=128))

                # ---- projection helpers ----
                pp = wctx.enter_context(tc.tile_pool(name="proj_ps", bufs=3,
                                                     space="PSUM"))

                def q_chunk(oj):
                    osl = slice(oj * 128, (oj + 1) * 128)
                    ps = pp.tile([128, TQ], f32, tag="proj", name=f"pjq{oj}")
                    for ki in range(KT):
                        nc.tensor.matmul(ps[:], weff["q"][:, ki, osl],
                                         xchunks[2][:, ki, :],
                                         start=(ki == 0), stop=(ki == KT - 1))
                    # qT = (ps + bq) * SCALE, fused on DVE
                    nc.vector.tensor_scalar(qT[:, oj, :], ps[:],
                                            bq_sb[:, oj:oj + 1], SCALE,
                                            ALU.add, ALU.mult)

                def k_chunk(oj, tch):
                    osl = slice(oj * 128, (oj + 1) * 128)
                    ps = pp.tile([128, 512], f32, tag="proj", name=f"pjk{oj}{tch}")
                    xc = xchunks[tch // 2]
                    xsl = slice((tch % 2) * 512, (tch % 2) * 512 + 512)
                    for ki in range(KT):
                        nc.tensor.matmul(ps[:], weff["k"][:, ki, osl],
                                         xc[:, ki, xsl],
                                         start=(ki == 0), stop=(ki == KT - 1))
                    nc.vector.tensor_scalar_add(
                        kT[:, oj, tch * 512:(tch + 1) * 512], ps[:],
                        bk_sb[:, oj:oj + 1])

                def v_tile(tt, nch):
                    nsl = slice(nch * 512, (nch + 1) * 512)
                    ps = pp.tile([128, 512], f32, tag="proj", name=f"pjv{tt}{nch}")
                    xc = xchunks[tt // 8]
                    xsl = slice((tt % 8) * 128, (tt % 8) * 128 + 128)
                    for ki in range(KT):
                        nc.tensor.matmul(ps[:], xc[:, ki, xsl],
                                         weff["v"][:, ki, nsl],
                                         start=(ki == 0), stop=(ki == KT - 1))
                    dst = vsb[:, tt, nch * 520:(nch + 1) * 520].rearrange(
                        "p (h c) -> p h c", c=65)[:, :, 0:64]
                    nc.scalar.copy(dst, ps[:].rearrange("p (h c) -> p h c", c=64))

                # ---- emission schedule ----
                weff_dma("k", sliced=True)
                x_dma(0, 0)
                x_dma(0, 1)
                weff_dma("o")
                weff_dma("v")
                x_dma(1, 0)
                x_dma(1, 1)
                bias_dma(0, 0)
                bias_dma(0, 1)
                bias_dma(1, 0)
                for ki in range(KT):
                    piece_lora("k", ki)
                # K projection over all 4 token chunks; o/v-weff LoRA adds ride
                # along (their DVE drains hide under the projection matmuls)
                lora_ov = [lambda ki=ki, p=p: piece_lora(p, ki)
                           for p in "ov" for ki in range(KT)]
                for tch in range(4):
                    for oj in range(KT):
                        k_chunk(oj, tch)
                        if oj % 2 == 1 and lora_ov:
                            lora_ov.pop(0)()
                while lora_ov:
                    lora_ov.pop(0)()
                weff_dma("q")  # reuses k's buffer once K-proj reads drain
                nc.gpsimd.dma_start(bo_row[:], bo_ap[:, :])  # cast f32->bf16
                lora_q = [lambda ki=ki: piece_lora("q", ki) for ki in range(KT)]
                for tt in range(8):
                    v_tile(tt, 0)
                    v_tile(tt, 1)
                x_dma(2, 0)  # query chunk: reuses chunk 0's buffer
                for tt in range(8, 16):
                    v_tile(tt, 0)
                    v_tile(tt, 1)
                    if lora_q:
                        lora_q.pop(0)()
                for oj in range(KT):
                    q_chunk(oj)

            # ================= attention =================
            ot_pool = rctx.enter_context(tc.tile_pool(name="ot", bufs=1))
            OT = ot_pool.tile([128, KT, TQ], bf16)
            ocat = ot_pool.tile([128, LT, D], bf16)

            with ExitStack() as actx:
                psS = actx.enter_context(tc.tile_pool(name="psS", bufs=3,
                                                      space="PSUM"))
                psO = actx.enter_context(tc.tile_pool(name="psO", bufs=2,
                                                      space="PSUM"))
                psF = actx.enter_context(tc.tile_pool(name="psF", bufs=2,
                                                      space="PSUM"))
                e_pool = actx.enter_context(tc.tile_pool(name="e", bufs=3))
                fin_pool = actx.enter_context(tc.tile_pool(name="fin", bufs=2))

                def attention(h):
                    hsl = slice((h % 2) * 64, (h % 2) * 64 + 64)
                    kj = h // 2
                    po = psO.tile([65, TQ], f32, tag="po", name=f"po{h}")
                    es = {}

                    def av(mi):
                        nc.tensor.matmul(po[:], vsb[:, mi, h * 65:h * 65 + 65],
                                         es.pop(mi)[:],
                                         start=(mi == 0), stop=(mi == MT - 1),
                                         skip_group_check=True)

                    for mi in range(MT):
                        bstage = bstages[(h, mi // 8)]
                        ps = psS.tile([128, TQ], f32, tag="ps")
                        nc.tensor.matmul(ps[:], kT[hsl, kj, mi * 128:(mi + 1) * 128],
                                         qT[hsl, kj, :], start=True, stop=False,
                                         skip_group_check=True)
                        nc.tensor.matmul(ps[:], ident[:], bstage[:, mi % 8, :],
                                         start=False, stop=True,
                                         skip_group_check=True)
                        e = e_pool.tile([128, TQ], bf16, tag="e")
                        nc.scalar.activation(e[:], ps[:], AF.Exp)
                        es[mi] = e
                        # AV lags by two m-tiles so the exp latency is hidden
                        if mi >= 2:
                            av(mi - 2)
                    av(MT - 2)
                    av(MT - 1)
                    del bstages[(h, 0)], bstages[(h, 1)]
                    if h + 1 < H:
                        bias_dma(h + 1, 1)
                    if h + 2 < H:
                        bias_dma(h + 2, 0)
                    # normalize into ocat columns h*64:(h+1)*64
                    stage = fin_pool.tile([65, TQ], f32, tag="st")
                    nc.vector.tensor_copy(stage[:, 0:256], po[:, 0:256])
                    nc.scalar.copy(stage[:, 256:512], po[:, 256:512])
                    for j in range(LT):
                        pf = psF.tile([128, 65], f32, tag="pf", name=f"pf{h}{j}")
                        nc.tensor.matmul(pf[:], stage[:, j * 128:(j + 1) * 128],
                                         identf[0:65, 0:65], is_transpose=True)
                        rec = fin_pool.tile([128, 1], f32, tag="rec")
                        nc.vector.reciprocal(rec[:], pf[:, 64:65])
                        nc.vector.tensor_scalar_mul(
                            ocat[:, j, h * 64:(h + 1) * 64], pf[:, 0:64], rec[:])

                def ot_piece(oj):
                    # transpose ocat block (heads 2oj, 2oj+1) into OT, add bv
                    osl = slice(oj * 128, (oj + 1) * 128)
                    for lt in range(LT):
                        pt = psF.tile([128, 128], bf16, tag="pf", name=f"pt{oj}{lt}")
                        nc.tensor.matmul(pt[:], ocat[:, lt, osl], ident[:],
                                         is_transpose=True)
                        nc.vector.tensor_scalar_add(
                            OT[:, oj, lt * 128:(lt + 1) * 128], pt[:],
                            bv_sb[:, oj:oj + 1])

                for h in range(H):
                    attention(h)
                    if h % 2 == 1:
                        ot_piece(h // 2)

                # ---- output projection ----
                y_pool = actx.enter_context(tc.tile_pool(name="ysb", bufs=4))
                for tt2 in range(QTT):
                    tsl = slice(tt2 * 128, (tt2 + 1) * 128)
                    for nch in range(2):
                        nsl = slice(nch * 512, (nch + 1) * 512)
                        ps = psS.tile([128, 512], f32, tag="ps",
                                      name=f"psy{tt2}{nch}")
                        for ki in range(KT):
                            nc.tensor.matmul(ps[:], OT[:, ki, tsl],
                                             woeff[:, ki, nsl],
                                             start=(ki == 0), stop=False,
                                             skip_group_check=True)
                        nc.tensor.matmul(ps[:], ones_row[:], bo_row[:, nsl],
                                         start=False, stop=True,
                                         skip_group_check=True)
                        ysb = y_pool.tile([128, 512], f32, tag="y")
                        # split the PSUM drain across ScalarE and DVE so the
                        # last chunks leave PSUM (and reach HBM) sooner
                        nc.scalar.copy(ysb[:, 0:256], ps[:, 0:256])
                        nc.vector.tensor_copy(ysb[:, 256:512], ps[:, 256:512])
                        nc.scalar.dma_start(
                            y_ap[tt2 * 128:(tt2 + 1) * 128, nsl], ysb[:])

    nc.compile()
    return nc


def _shard_inputs(inputs):
    import ml_dtypes
    bf16 = np.dtype(ml_dtypes.bfloat16)

    x = np.asarray(inputs["x"])  # [B, L, D] f32
    xbf = x.astype(bf16)
    # biasT_full: [H, key m, query l] bf16
    biasT_full = np.ascontiguousarray(
        np.swapaxes(np.asarray(inputs["attn_bias"])[0], 1, 2)).astype(bf16)

    shared = {}
    ats = np.zeros((32, 2, D), np.float32)
    bs = np.zeros((32, 4, D), np.float32)
    for i, p in enumerate("qkvo"):
        shared[f"WT{p}"] = np.ascontiguousarray(
            np.asarray(inputs[f"W{p}"]).T).astype(bf16)
        rsl = slice((i % 2) * R, (i % 2) * R + R)
        ats[rsl, i // 2] = np.asarray(inputs[f"A{p}"]).T
        bs[rsl, i] = np.asarray(inputs[f"B{p}"])
    shared["ATs"] = ats.astype(bf16)
    shared["Bs"] = bs.astype(bf16)
    for p in "qkv":
        shared[f"b{p}"] = np.asarray(
            inputs[f"b{p}"], np.float32).reshape(D, 1)
    shared["bo"] = np.asarray(inputs["bo"], np.float32).reshape(1, D)

    xbT = [np.ascontiguousarray(xbf[b].T) for b in range(B)]  # [D, L] each
    in_maps = []
    for c in range(N_CORES):
        b, qc = divmod(c, 4)
        qsl = slice(qc * TQ, (qc + 1) * TQ)
        m = dict(shared)
        m["xbT"] = xbT[b]
        m["xqT"] = np.ascontiguousarray(xbT[b][:, qsl])
        m["biasT"] = np.ascontiguousarray(biasT_full[:, :, qsl])
        in_maps.append(m)
    return in_maps


def _gather_outputs(results):
    y = np.empty((B, L, D), np.float32)
    for c in range(N_CORES):
        b, qc = divmod(c, 4)
        y[b, qc * TQ:(qc + 1) * TQ] = results[c]["y"]
    return y


def get_nc(**kw):
    key = ("nc", tuple(sorted(kw.items())))
    if key not in _CACHE:
        _CACHE[key] = _build_kernel(**kw)
    return _CACHE[key]


def _get_runner():
    """Cached jitted SPMD executable (avoids re-tracing on repeated calls)."""
    if "runner" in _CACHE:
        return _CACHE["runner"]
    import jax
    from jax.sharding import Mesh, PartitionSpec
    from jax.experimental.shard_map import shard_map
    import concourse.mybir as mybir
    from concourse.bass2jax import (_bass_exec_p, install_neuronx_cc_hook,
                                    partition_id_tensor)

    nc = get_nc()
    install_neuronx_cc_hook()
    partition_name = nc.partition_id_tensor.name if nc.partition_id_tensor else None
    in_names, out_names, out_avals, zero_outs = [], [], [], []
    for alloc in nc.m.functions[0].allocations:
        if not isinstance(alloc, mybir.MemoryLocationSet):
            continue
        name = alloc.memorylocations[0].name
        if alloc.kind == "ExternalInput":
            if name != partition_name:
                in_names.append(name)
        elif alloc.kind == "ExternalOutput":
            shape = tuple(alloc.tensor_shape)
            dtype = mybir.dt.np(alloc.dtype)
            out_names.append(name)
            out_avals.append(jax.core.ShapedArray(shape, dtype))
            zero_outs.append(np.zeros(shape, dtype))
    n_params = len(in_names)
    n_outs = len(out_avals)
    all_in_names = list(in_names) + list(out_names)
    if partition_name is not None:
        all_in_names.append(partition_name)

    def _body(*args):
        operands = list(args)
        if partition_name is not None:
            operands.append(partition_id_tensor())
        outs = _bass_exec_p.bind(
            *operands,
            out_avals=tuple(out_avals),
            in_names=tuple(all_in_names),
            out_names=tuple(out_names),
            lowering_input_output_aliases=(),
            sim_require_finite=True,
            sim_require_nnan=True,
            nc=nc,
        )
        return tuple(outs)

    devices = jax.devices()[:N_CORES]
    mesh = Mesh(np.asarray(devices), ("core",))
    in_specs = (PartitionSpec("core"),) * (n_params + n_outs)
    out_specs = (PartitionSpec("core"),) * n_outs
    fn = jax.jit(shard_map(_body, mesh=mesh, in_specs=in_specs,
                           out_specs=out_specs, check_rep=False),
                 keep_unused=True)
    _CACHE["runner"] = (fn, in_names, out_names, zero_outs)
    return _CACHE["runner"]


def run_on_device(in_maps):
    import jax
    fn, in_names, out_names, zero_outs = _get_runner()
    concat_in = [np.concatenate([np.asarray(in_maps[c][nm])
                                 for c in range(N_CORES)], axis=0)
                 for nm in in_names]
    concat_zeros = [np.zeros((N_CORES * z.shape[0], *z.shape[1:]), z.dtype)
                    for z in zero_outs]
    out = fn(*concat_in, *concat_zeros)
    jax.block_until_ready(out)
    results = []
    for c in range(N_CORES):
        d = {}
        for i, nm in enumerate(out_names):
            arr = np.asarray(out[i])
            per = arr.shape[0] // N_CORES
            d[nm] = arr[c * per:(c + 1) * per]
        results.append(d)
    return results


def kernel(**inputs) -> np.ndarray:
    in_maps = _shard_inputs(inputs)
    results = run_on_device(in_maps)
    return _gather_outputs(results)


# revision 43
# speedup vs baseline: 17023.0264x; 1.0768x over previous
"""Multi-head self-attention with LoRA projections on 8 Trainium2 NeuronCores.

Problem: nn_MultiHeadSelfAttention (B=2, L=2048, D=1024, H=16, hd=64, LoRA r=16).

Sharding (communication-free): core c owns batch b = c//4 and query rows
qo = (c%4)*512 of that batch. Each core computes K/V projections for its
whole batch (replicated across the 4 cores sharing the batch), Q for its
512 rows, attention for all 16 heads over those rows, and the output
projection for those rows. The final output is a clean concat across cores —
no collectives and no host-side reduction.

Host-side prep is layout/dtype marshaling only: inputs are cast to bf16
(numerically identical to the on-device cast-DMA the previous version used),
W is shipped pre-transposed as [in, out], and attn_bias is shipped
pre-transposed per head as [key m, query l] so it can be injected into the
S^T PSUM accumulation by a single stationary-identity matmul per tile.
LoRA folding (W_eff = W^T + 0.5*A@B) happens on device.

Per-core pipeline (bf16 on the PE, fp32 accumulation in PSUM):
  1. W_eff[p] [128, 8, 1024] (in-chunk part, ki, out) loaded by DMA; LoRA
     term 0.5*A@B added via K=16 matmuls into PSUM + DVE add.
  2. x tiles PE-transposed into xT chunks [128, 8, 1024] (2 chunks of batch
     tokens for K/V, then this core's 512 query tokens reusing chunk 0's
     buffer).
  3. kT [128, 8, 2048] / qT [128, 8, 512] in [out, token] layout (+bias;
     attn scale folded into the Q PSUM->SBUF copy); v natural [token, d]
     in vsb with a ones column per head (softmax row sums); bv deferred
     past softmax (softmax rows sum to 1).
  4. Per head: S^T tiles [m=128, l=512] in PSUM (K=64 matmul), pre-transposed
     bias injected by matmul(ident, biasT_tile); exp on ScalarE -> E^T bf16;
     AV with lhsT=[v|1] accumulates O'^T [65, 512] (row 64 = denominator);
     normalize via [65]->[128] PE transpose + DVE reciprocal into ocat.
  5. OT [128, 8, 512] (d, token) via PE transposes (+bv); output projection
     y = OT^T @ Wo_eff + bo for this core's 512 rows.
"""

import numpy as np

B = 2
L = 2048
D = 1024
H = 16
HD = 64
R = 16
SCALING = 0.5  # LoRA alpha/r
SCALE = HD ** (-0.5)  # attention scale, applied in the Q PSUM->SBUF copy

N_CORES = 8
TQ = 512  # query rows per core
LT = TQ // 128  # 4 query tiles
MT = L // 128  # 16 key tiles
KT = D // 128  # 8 contraction chunks
NTT = L // 128  # 16 batch-token tiles
QTT = TQ // 128  # 4 query-token tiles

_CACHE = {}


def _build_kernel(num_devices=N_CORES, repeat=1):
    import concourse.tile as tile
    import concourse.mybir as mybir
    from concourse import bacc
    from concourse.masks import make_identity
    from contextlib import ExitStack

    f32 = mybir.dt.float32
    bf16 = mybir.dt.bfloat16
    AF = mybir.ActivationFunctionType
    ALU = mybir.AluOpType

    nc = bacc.Bacc("TRN2", target_bir_lowering=False, debug=False,
                   enable_asserts=False, num_devices=num_devices)

    # ---- per-core external inputs (bf16 unless noted) ----
    xbT_ap = nc.dram_tensor("xbT", [D, L], bf16, kind="ExternalInput").ap()
    xqT_ap = nc.dram_tensor("xqT", [D, TQ], bf16, kind="ExternalInput").ap()
    biasT_ap = nc.dram_tensor("biasT", [H, L, TQ], bf16, kind="ExternalInput").ap()
    wt_aps = {}
    for p in "qkvo":
        wt_aps[p] = nc.dram_tensor(f"WT{p}", [D, D], bf16, kind="ExternalInput").ap()
    # LoRA factors packed into 32 partition rows (base partition 0) with the
    # projection selected by free-dim blocks + zero padding:
    #   ATs[0:16, 0] = Aq^T, ATs[16:32, 0] = Ak^T,
    #   ATs[0:16, 1] = Av^T, ATs[16:32, 1] = Ao^T;
    #   Bs[:, i] = B of projection i ("qkvo") in the row range matching its
    #   A block, zeros elsewhere — so a K=32 matmul picks out one projection.
    ats_ap = nc.dram_tensor("ATs", [32, 2, D], bf16, kind="ExternalInput").ap()
    bs_ap = nc.dram_tensor("Bs", [32, 4, D], bf16, kind="ExternalInput").ap()
    bq_ap = nc.dram_tensor("bq", [D, 1], f32, kind="ExternalInput").ap()
    bk_ap = nc.dram_tensor("bk", [D, 1], f32, kind="ExternalInput").ap()
    bv_ap = nc.dram_tensor("bv", [D, 1], f32, kind="ExternalInput").ap()
    bo_ap = nc.dram_tensor("bo", [1, D], f32, kind="ExternalInput").ap()

    y_ap = nc.dram_tensor("y", [TQ, D], f32, kind="ExternalOutput").ap()

    with tile.TileContext(nc) as tc, ExitStack() as top:
        const_pool = top.enter_context(tc.tile_pool(name="const", bufs=1))
        ident = const_pool.tile([128, 128], bf16)
        make_identity(nc, ident[:])
        identf = const_pool.tile([128, 128], f32)
        make_identity(nc, identf[:])
        ones_row = const_pool.tile([1, 128], bf16)
        nc.gpsimd.memset(ones_row[:], 1.0)

        for rep in range(repeat):
          with ExitStack() as rctx:
            cpool = rctx.enter_context(tc.tile_pool(name="cvec", bufs=1))
            bq_sb = cpool.tile([128, KT], f32)
            bk_sb = cpool.tile([128, KT], f32)
            bv_sb = cpool.tile([128, KT], f32)
            bo_row = cpool.tile([1, D], bf16)
            nc.scalar.dma_start(bq_sb[:], bq_ap.rearrange("(oj p) one -> p (oj one)", p=128))
            nc.scalar.dma_start(bk_sb[:], bk_ap.rearrange("(oj p) one -> p (oj one)", p=128))
            nc.scalar.dma_start(bv_sb[:], bv_ap.rearrange("(oj p) one -> p (oj one)", p=128))

            # ---- persistent tiles for this rep ----
            kqv_pool = rctx.enter_context(tc.tile_pool(name="kqv", bufs=1))
            kT = kqv_pool.tile([128, KT, L], bf16)
            qT = kqv_pool.tile([128, KT, TQ], bf16)
            vsb = kqv_pool.tile([128, NTT, H * 65], bf16)
            woeff_pool = rctx.enter_context(tc.tile_pool(name="woeff", bufs=1))
            woeff = woeff_pool.tile([128, KT, D], bf16)
            for h in range(H):
                nc.vector.memset(vsb[:, :, h * 65 + 64:h * 65 + 65], 1.0)

            # attn bias staged per (head, half): [128, 8 m-tiles, 512]
            bias_pool = rctx.enter_context(tc.tile_pool(name="biasT", bufs=4))
            bstages = {}

            def bias_dma(h, half):
                bt = bias_pool.tile([128, MT // 2, TQ], bf16, tag="bT",
                                    name=f"bT{h}_{half}")
                nc.sync.dma_start(
                    bt[:],
                    biasT_ap[h][half * 1024:(half + 1) * 1024, :].rearrange(
                        "(mt p) l -> p mt l", p=128))
                bstages[(h, half)] = bt

            with ExitStack() as wctx:
                lora_sm = wctx.enter_context(tc.tile_pool(name="lora_sm", bufs=1))
                wps = wctx.enter_context(
                    tc.tile_pool(name="wps", bufs=2, space="PSUM"))
                weff_pool = wctx.enter_context(tc.tile_pool(name="weff", bufs=2))

                ats_raw = lora_sm.tile([32, 2, D], bf16)
                bs = lora_sm.tile([32, 4, D], bf16)
                ats = lora_sm.tile([32, 2, D], bf16)
                nc.sync.dma_start(ats_raw[:], ats_ap[:, :, :])
                nc.sync.dma_start(bs[:], bs_ap[:, :, :])
                nc.vector.tensor_scalar_mul(ats[:], ats_raw[:], SCALING)
                PI = {"q": 0, "k": 1, "v": 2, "o": 3}

                weff = {}
                weff["o"] = woeff

                def weff_dma(p, sliced=False):
                    if p != "o":
                        weff[p] = weff_pool.tile([128, KT, D], bf16, tag="weff",
                                                 name=f"weff_{p}")
                    src = wt_aps[p].rearrange("(ki p2) o -> p2 ki o", p2=128)
                    if sliced:
                        # per-ki slices so the LoRA adds pipeline behind the DMA
                        for ki in range(KT):
                            nc.sync.dma_start(
                                weff[p][:, ki:ki + 1, :], src[:, ki:ki + 1, :])
                    else:
                        nc.sync.dma_start(weff[p][:], src)

                def piece_lora(p, ki):
                    # weff[p][:, ki, :] += SCALING * A[ki-chunk, :] @ B
                    # (K=32 matmul; zero rows in the Bs block mask the paired
                    # projection sharing the lhsT rows)
                    pi = PI[p]
                    ksl = slice(ki * 128, (ki + 1) * 128)
                    ps = wps.tile([128, D], f32, tag="wps", name=f"lo{p}{ki}")
                    for half in range(2):
                        osl = slice(half * 512, (half + 1) * 512)
                        nc.tensor.matmul(ps[:, osl], ats[:, pi // 2, ksl],
                                         bs[:, pi, osl],
                                         start=True, stop=True,
                                         skip_group_check=True)
                    nc.vector.tensor_tensor(weff[p][:, ki, :], weff[p][:, ki, :],
                                            ps[:], ALU.add)

                # ---- x chunks DMA'd directly (host ships x pre-transposed) --
                xt_pool = wctx.enter_context(tc.tile_pool(name="xT", bufs=2))
                xchunks = {}

                def x_dma(cidx, half):
                    # chunk 0: batch tokens 0-1023, chunk 1: 1024-2047,
                    # chunk 2: the core's 512 query tokens (reuses buffer 0)
                    if cidx == 2:
                        xchunks[2] = xt_pool.tile([128, KT, TQ], bf16,
                                                  tag="xc", name="xc2")
                        nc.sync.dma_start(
                            xchunks[2][:],
                            xqT_ap.rearrange("(ki p) t -> p ki t", p=128))
                        return
                    if half == 0:
                        xchunks[cidx] = xt_pool.tile([128, KT, 1024], bf16,
                                                     tag="xc", name=f"xc{cidx}")
                    tof = cidx * 1024 + half * 512
                    nc.sync.dma_start(
                        xchunks[cidx][:, :, half * 512:(half + 1) * 512],
                        xbT_ap[:, tof:tof + 512].rearrange(
                            "(ki p) t -> p ki t", p=128))

                # ---- projection helpers ----
                pp = wctx.enter_context(tc.tile_pool(name="proj_ps", bufs=3,
                                                     space="PSUM"))

                def q_chunk(oj):
                    osl = slice(oj * 128, (oj + 1) * 128)
                    ps = pp.tile([128, TQ], f32, tag="proj", name=f"pjq{oj}")
                    for ki in range(KT):
                        nc.tensor.matmul(ps[:], weff["q"][:, ki, osl],
                                         xchunks[2][:, ki, :],
                                         start=(ki == 0), stop=(ki == KT - 1))
                    # qT = (ps + bq) * SCALE, fused on DVE
                    nc.vector.tensor_scalar(qT[:, oj, :], ps[:],
                                            bq_sb[:, oj:oj + 1], SCALE,
                                            ALU.add, ALU.mult)

                def k_chunk(oj, tch):
                    osl = slice(oj * 128, (oj + 1) * 128)
                    ps = pp.tile([128, 512], f32, tag="proj", name=f"pjk{oj}{tch}")
                    xc = xchunks[tch // 2]
                    xsl = slice((tch % 2) * 512, (tch % 2) * 512 + 512)
                    for ki in range(KT):
                        nc.tensor.matmul(ps[:], weff["k"][:, ki, osl],
                                         xc[:, ki, xsl],
                                         start=(ki == 0), stop=(ki == KT - 1))
                    nc.vector.tensor_scalar_add(
                        kT[:, oj, tch * 512:(tch + 1) * 512], ps[:],
                        bk_sb[:, oj:oj + 1])

                def v_tile(tt, nch):
                    nsl = slice(nch * 512, (nch + 1) * 512)
                    ps = pp.tile([128, 512], f32, tag="proj", name=f"pjv{tt}{nch}")
                    xc = xchunks[tt // 8]
                    xsl = slice((tt % 8) * 128, (tt % 8) * 128 + 128)
                    for ki in range(KT):
                        nc.tensor.matmul(ps[:], xc[:, ki, xsl],
                                         weff["v"][:, ki, nsl],
                                         start=(ki == 0), stop=(ki == KT - 1))
                    dst = vsb[:, tt, nch * 520:(nch + 1) * 520].rearrange(
                        "p (h c) -> p h c", c=65)[:, :, 0:64]
                    nc.scalar.copy(dst, ps[:].rearrange("p (h c) -> p h c", c=64))

                # ---- emission schedule ----
                weff_dma("k", sliced=True)
                x_dma(0, 0)
                x_dma(0, 1)
                weff_dma("o")
                weff_dma("v")
                x_dma(1, 0)
                x_dma(1, 1)
                bias_dma(0, 0)
                bias_dma(0, 1)
                bias_dma(1, 0)
                bias_dma(1, 1)
                for ki in range(KT):
                    piece_lora("k", ki)
                # K projection over all 4 token chunks; o/v-weff LoRA adds ride
                # along (their DVE drains hide under the projection matmuls)
                lora_ov = [lambda ki=ki, p=p: piece_lora(p, ki)
                           for p in "ov" for ki in range(KT)]
                for tch in range(4):
                    for oj in range(KT):
                        k_chunk(oj, tch)
                        if oj % 2 == 1 and lora_ov:
                            lora_ov.pop(0)()
                while lora_ov:
                    lora_ov.pop(0)()
                weff_dma("q")  # reuses k's buffer once K-proj reads drain
                nc.gpsimd.dma_start(bo_row[:], bo_ap[:, :])  # cast f32->bf16
                lora_q = [lambda ki=ki: piece_lora("q", ki) for ki in range(KT)]
                for tt in range(8):
                    v_tile(tt, 0)
                    v_tile(tt, 1)
                x_dma(2, 0)  # query chunk: reuses chunk 0's buffer
                for tt in range(8, 16):
                    v_tile(tt, 0)
                    v_tile(tt, 1)
                    if lora_q:
                        lora_q.pop(0)()
                for oj in range(KT):
                    q_chunk(oj)

            # ================= attention =================
            ot_pool = rctx.enter_context(tc.tile_pool(name="ot", bufs=1))
            OT = ot_pool.tile([128, KT, TQ], bf16)
            ocat = ot_pool.tile([128, LT, D], bf16)

            with ExitStack() as actx:
                psS = actx.enter_context(tc.tile_pool(name="psS", bufs=3,
                                                      space="PSUM"))
                psO = actx.enter_context(tc.tile_pool(name="psO", bufs=2,
                                                      space="PSUM"))
                psF = actx.enter_context(tc.tile_pool(name="psF", bufs=2,
                                                      space="PSUM"))
                e_pool = actx.enter_context(tc.tile_pool(name="e", bufs=3))
                fin_pool = actx.enter_context(tc.tile_pool(name="fin", bufs=2))

                def attention(h):
                    hsl = slice((h % 2) * 64, (h % 2) * 64 + 64)
                    kj = h // 2
                    po = psO.tile([65, TQ], f32, tag="po", name=f"po{h}")
                    es = {}

                    def av(mi):
                        nc.tensor.matmul(po[:], vsb[:, mi, h * 65:h * 65 + 65],
                                         es.pop(mi)[:],
                                         start=(mi == 0), stop=(mi == MT - 1),
                                         skip_group_check=True)

                    for mi in range(MT):
                        bstage = bstages[(h, mi // 8)]
                        ps = psS.tile([128, TQ], f32, tag="ps")
                        nc.tensor.matmul(ps[:], kT[hsl, kj, mi * 128:(mi + 1) * 128],
                                         qT[hsl, kj, :], start=True, stop=False,
                                         skip_group_check=True)
                        nc.tensor.matmul(ps[:], ident[:], bstage[:, mi % 8, :],
                                         start=False, stop=True,
                                         skip_group_check=True)
                        e = e_pool.tile([128, TQ], bf16, tag="e")
                        nc.scalar.activation(e[:], ps[:], AF.Exp)
                        es[mi] = e
                        # AV lags by two m-tiles so the exp latency is hidden
                        if mi >= 2:
                            av(mi - 2)
                    av(MT - 2)
                    av(MT - 1)
                    del bstages[(h, 0)], bstages[(h, 1)]
                    if h + 2 < H:
                        bias_dma(h + 2, 0)
                        bias_dma(h + 2, 1)
                    # normalize into ocat columns h*64:(h+1)*64
                    stage = fin_pool.tile([65, TQ], f32, tag="st")
                    nc.vector.tensor_copy(stage[:, 0:256], po[:, 0:256])
                    nc.scalar.copy(stage[:, 256:512], po[:, 256:512])
                    for j in range(LT):
                        pf = psF.tile([128, 65], f32, tag="pf", name=f"pf{h}{j}")
                        nc.tensor.matmul(pf[:], stage[:, j * 128:(j + 1) * 128],
                                         identf[0:65, 0:65], is_transpose=True)
                        rec = fin_pool.tile([128, 1], f32, tag="rec")
                        nc.vector.reciprocal(rec[:], pf[:, 64:65])
                        nc.vector.tensor_scalar_mul(
                            ocat[:, j, h * 64:(h + 1) * 64], pf[:, 0:64], rec[:])

                def ot_piece(oj):
                    # transpose ocat block (heads 2oj, 2oj+1) into OT, add bv
                    osl = slice(oj * 128, (oj + 1) * 128)
                    for lt in range(LT):
                        pt = psF.tile([128, 128], bf16, tag="pf", name=f"pt{oj}{lt}")
                        nc.tensor.matmul(pt[:], ocat[:, lt, osl], ident[:],
                                         is_transpose=True)
                        nc.vector.tensor_scalar_add(
                            OT[:, oj, lt * 128:(lt + 1) * 128], pt[:],
                            bv_sb[:, oj:oj + 1])

                for h in range(H):
                    attention(h)
                    if h % 2 == 1:
                        ot_piece(h // 2)

                # ---- output projection ----
                y_pool = actx.enter_context(tc.tile_pool(name="ysb", bufs=4))
                for tt2 in range(QTT):
                    tsl = slice(tt2 * 128, (tt2 + 1) * 128)
                    for nch in range(2):
                        nsl = slice(nch * 512, (nch + 1) * 512)
                        ps = psS.tile([128, 512], f32, tag="ps",
                                      name=f"psy{tt2}{nch}")
                        for ki in range(KT):
                            nc.tensor.matmul(ps[:], OT[:, ki, tsl],
                                             woeff[:, ki, nsl],
                                             start=(ki == 0), stop=False,
                                             skip_group_check=True)
                        nc.tensor.matmul(ps[:], ones_row[:], bo_row[:, nsl],
                                         start=False, stop=True,
                                         skip_group_check=True)
                        ysb = y_pool.tile([128, 512], f32, tag="y")
                        # split the PSUM drain across ScalarE and DVE so the
                        # last chunks leave PSUM (and reach HBM) sooner
                        nc.scalar.copy(ysb[:, 0:256], ps[:, 0:256])
                        nc.vector.tensor_copy(ysb[:, 256:512], ps[:, 256:512])
                        nc.scalar.dma_start(
                            y_ap[tt2 * 128:(tt2 + 1) * 128, nsl], ysb[:])

    nc.compile()
    return nc


def _shard_inputs(inputs):
    import ml_dtypes
    bf16 = np.dtype(ml_dtypes.bfloat16)

    x = np.asarray(inputs["x"])  # [B, L, D] f32
    xbf = x.astype(bf16)
    # biasT_full: [H, key m, query l] bf16
    biasT_full = np.ascontiguousarray(
        np.swapaxes(np.asarray(inputs["attn_bias"])[0], 1, 2)).astype(bf16)

    shared = {}
    ats = np.zeros((32, 2, D), np.float32)
    bs = np.zeros((32, 4, D), np.float32)
    for i, p in enumerate("qkvo"):
        shared[f"WT{p}"] = np.ascontiguousarray(
            np.asarray(inputs[f"W{p}"]).T).astype(bf16)
        rsl = slice((i % 2) * R, (i % 2) * R + R)
        ats[rsl, i // 2] = np.asarray(inputs[f"A{p}"]).T
        bs[rsl, i] = np.asarray(inputs[f"B{p}"])
    shared["ATs"] = ats.astype(bf16)
    shared["Bs"] = bs.astype(bf16)
    for p in "qkv":
        shared[f"b{p}"] = np.asarray(
            inputs[f"b{p}"], np.float32).reshape(D, 1)
    shared["bo"] = np.asarray(inputs["bo"], np.float32).reshape(1, D)

    xbT = [np.ascontiguousarray(xbf[b].T) for b in range(B)]  # [D, L] each
    in_maps = []
    for c in range(N_CORES):
        b, qc = divmod(c, 4)
        qsl = slice(qc * TQ, (qc + 1) * TQ)
        m = dict(shared)
        m["xbT"] = xbT[b]
        m["xqT"] = np.ascontiguousarray(xbT[b][:, qsl])
        m["biasT"] = np.ascontiguousarray(biasT_full[:, :, qsl])
        in_maps.append(m)
    return in_maps


def _gather_outputs(results):
    y = np.empty((B, L, D), np.float32)
    for c in range(N_CORES):
        b, qc = divmod(c, 4)
        y[b, qc * TQ:(qc + 1) * TQ] = results[c]["y"]
    return y


def get_nc(**kw):
    key = ("nc", tuple(sorted(kw.items())))
    if key not in _CACHE:
        _CACHE[key] = _build_kernel(**kw)
    return _CACHE[key]


def _get_runner():
    """Cached jitted SPMD executable (avoids re-tracing on repeated calls)."""
    if "runner" in _CACHE:
        return _CACHE["runner"]
    import jax
    from jax.sharding import Mesh, PartitionSpec
    from jax.experimental.shard_map import shard_map
    import concourse.mybir as mybir
    from concourse.bass2jax import (_bass_exec_p, install_neuronx_cc_hook,
                                    partition_id_tensor)

    nc = get_nc()
    install_neuronx_cc_hook()
    partition_name = nc.partition_id_tensor.name if nc.partition_id_tensor else None
    in_names, out_names, out_avals, zero_outs = [], [], [], []
    for alloc in nc.m.functions[0].allocations:
        if not isinstance(alloc, mybir.MemoryLocationSet):
            continue
        name = alloc.memorylocations[0].name
        if alloc.kind == "ExternalInput":
            if name != partition_name:
                in_names.append(name)
        elif alloc.kind == "ExternalOutput":
            shape = tuple(alloc.tensor_shape)
            dtype = mybir.dt.np(alloc.dtype)
            out_names.append(name)
            out_avals.append(jax.core.ShapedArray(shape, dtype))
            zero_outs.append(np.zeros(shape, dtype))
    n_params = len(in_names)
    n_outs = len(out_avals)
    all_in_names = list(in_names) + list(out_names)
    if partition_name is not None:
        all_in_names.append(partition_name)

    def _body(*args):
        operands = list(args)
        if partition_name is not None:
            operands.append(partition_id_tensor())
        outs = _bass_exec_p.bind(
            *operands,
            out_avals=tuple(out_avals),
            in_names=tuple(all_in_names),
            out_names=tuple(out_names),
            lowering_input_output_aliases=(),
            sim_require_finite=True,
            sim_require_nnan=True,
            nc=nc,
        )
        return tuple(outs)

    devices = jax.devices()[:N_CORES]
    mesh = Mesh(np.asarray(devices), ("core",))
    in_specs = (PartitionSpec("core"),) * (n_params + n_outs)
    out_specs = (PartitionSpec("core"),) * n_outs
    fn = jax.jit(shard_map(_body, mesh=mesh, in_specs=in_specs,
                           out_specs=out_specs, check_rep=False),
                 keep_unused=True)
    _CACHE["runner"] = (fn, in_names, out_names, zero_outs)
    return _CACHE["runner"]


def run_on_device(in_maps):
    import jax
    fn, in_names, out_names, zero_outs = _get_runner()
    concat_in = [np.concatenate([np.asarray(in_maps[c][nm])
                                 for c in range(N_CORES)], axis=0)
                 for nm in in_names]
    concat_zeros = [np.zeros((N_CORES * z.shape[0], *z.shape[1:]), z.dtype)
                    for z in zero_outs]
    out = fn(*concat_in, *concat_zeros)
    jax.block_until_ready(out)
    results = []
    for c in range(N_CORES):
        d = {}
        for i, nm in enumerate(out_names):
            arr = np.asarray(out[i])
            per = arr.shape[0] // N_CORES
            d[nm] = arr[c * per:(c + 1) * per]
        results.append(d)
    return results


def kernel(**inputs) -> np.ndarray:
    in_maps = _shard_inputs(inputs)
    results = run_on_device(in_maps)
    return _gather_outputs(results)
